# revision 1
# baseline (speedup 1.0000x reference)
"""Trainium2 Bass kernel for the MFVI second-order CRF message-passing module.

Math (per batch element, per iteration):
    q_sm = softmax(q, axis=-1)                               # over T=256
    Lj   = q_sm @ Tj          (j=1,2)
    Rj   = q_sm @ Tj.T        (j=1,2)
    msg[s] = L1[s-1] + L2[s-2] + R1[s+1] + R2[s+2]           # 0 outside [0,S)
    q    = (unary + msg + start/end-corrections) * mask

Device strategy (8 cores, data-parallel over batch B=32 -> 4/core):
  * Everything stored transposed: [T(2x128 partitions), S(free)] per batch
    element, so the +-1/+-2 sequence shifts become free-dim offsets of the
    matmul's moving operand, accumulated directly into PSUM (msg is never
    materialized wide).
  * softmax over T = partition reduction: ones[128x128] matmul gives the
    column sums broadcast to all partitions in one pass; DVE reciprocal +
    one multiply normalizes.
  * start/end transition scatter-adds are folded on the HOST into a
    corrected unary' (they always land at unmasked positions); the raw
    unary is shipped separately as the softmax-chain seed.
  * matmuls run in float32r (full PE rate); everything else fp32.
"""
import os
import sys

sys.path.insert(0, "/opt/trn_rl_repo")

import numpy as np

import concourse.mybir as mybir
from concourse.bass import Bass
from concourse.tile import TileContext
from concourse import bass_utils

B, S, T = 32, 1024, 256
WINDOW = 2
ITERS = 3
N_CORES = 8
BPC = B // N_CORES          # batch elems per core
NCH = T // 128              # partition chunks of T
HALF = S // 2               # masked half starts here (lengths >= S//2)

# matmul dtype: float32r streams 1 row/cycle (4x faster than float32) at
# ~tf32-ish precision; set MFVI_FP32=1 to force exact-rate fp32.
MM_DT = mybir.dt.float32 if os.environ.get("MFVI_FP32") else mybir.dt.float32r


def _split_sync_waits(nc):
    """walrus in this env accepts at most ONE sync wait per instruction;
    Tile emits several. Move extras onto same-engine NoOps inserted just
    before the offending instruction."""
    ctr = 0
    for f in nc.m.functions:
        for block in f.blocks:
            out = []
            changed = False
            for inst in block.instructions:
                si = inst.sync_info
                waits = list(si.on_wait) if si is not None and si.on_wait else []
                if len(waits) > 1:
                    changed = True
                    for w in waits[:-1]:
                        ctr += 1
                        nop = mybir.InstNoOp(
                            name=f"I-waitsplit-{ctr}",
                            engine=inst.engine, ins=[], outs=[])
                        nop.sync_info = mybir.SyncInfo(on_wait=[w], on_update=[])
                        out.append(nop)
                    si.on_wait = [waits[-1]]
                    inst.sync_info = si
                out.append(inst)
            if changed:
                block.instructions = out
    return nc


def _build():
    f32 = mybir.dt.float32
    nc = Bass(trn_type="TRN2", target_bir_lowering=False, debug=False,
              num_devices=N_CORES)

    u_raw = nc.dram_tensor("u_raw", [BPC, NCH, 128, S], f32,
                           kind="ExternalInput").ap()
    u_corr = nc.dram_tensor("u_corr", [BPC, NCH, 128, S], f32,
                            kind="ExternalInput").ap()
    wmat = nc.dram_tensor("wmat", [128, 4 * NCH * NCH * 128], f32,
                          kind="ExternalInput").ap()
    maskbc = nc.dram_tensor("maskbc", [BPC, 128, HALF], f32,
                            kind="ExternalInput").ap()
    qout = nc.dram_tensor("qout", [BPC, NCH, 128, S], f32,
                          kind="ExternalOutput").ap()

    with TileContext(nc) as tc:
        with tc.tile_pool(name="persist", bufs=1) as pp, \
             tc.tile_pool(name="work", bufs=2) as wp, \
             tc.tile_pool(name="psum", bufs=2, space="PSUM") as psp:

            # ---------- setup ----------
            q = [[pp.tile([128, S], f32, tag=f"q{n}_{c}", name=f"q{n}_{c}") for c in range(NCH)]
                 for n in range(BPC)]
            up = [[pp.tile([128, S], f32, tag=f"up{n}_{c}", name=f"up{n}_{c}") for c in range(NCH)]
                  for n in range(BPC)]
            wst = pp.tile([128, 4 * NCH * NCH * 128], f32, tag="wst", name="wst")

            # DMA order: batch 0 and the weights first so compute starts
            # while the rest of the batch streams in (HBM-bandwidth bound).
            for c in range(NCH):
                nc.sync.dma_start(out=q[0][c], in_=u_raw[0, c])
            nc.sync.dma_start(out=wst, in_=wmat)
            for c in range(NCH):
                nc.sync.dma_start(out=up[0][c], in_=u_corr[0, c])
            for n in range(1, BPC):
                for c in range(NCH):
                    nc.sync.dma_start(out=q[n][c], in_=u_raw[n, c])
                for c in range(NCH):
                    nc.sync.dma_start(out=up[n][c], in_=u_corr[n, c])

            wmm = pp.tile([128, 4 * NCH * NCH * 128], MM_DT, tag="wmm", name="wmm")

            ones_f = pp.tile([128, 128], f32, tag="ones_f", name="ones_f")
            nc.vector.memset(ones_f[:], 1.0)
            ones_m = pp.tile([128, 128], MM_DT, tag="ones_m", name="ones_m")
            nc.vector.tensor_copy(out=ones_m[:], in_=ones_f[:])

            # persistent softmax tiles, padded with WINDOW zero guard
            # columns both sides so shifted conv matmuls never read OOB
            zer = pp.tile([128, WINDOW], f32, tag="zer", name="zer")
            nc.vector.memset(zer[:], 0.0)
            SP = S + 2 * WINDOW
            qsm = [[pp.tile([128, SP], MM_DT, tag=f"qsm{n}_{c}",
                            name=f"qsm{n}_{c}") for c in range(NCH)]
                   for n in range(BPC)]
            for n in range(BPC):
                for c in range(NCH):
                    nc.vector.tensor_copy(out=qsm[n][c][:, 0:WINDOW],
                                          in_=zer[:])
                    nc.vector.tensor_copy(out=qsm[n][c][:, S + WINDOW:],
                                          in_=zer[:])
            nc.vector.tensor_copy(out=wmm[:], in_=wst[:])

            # mask, pre-broadcast on host, second half of S only (first
            # half is always unmasked: lengths >= S/2)
            mb = []
            for n in range(BPC):
                t = pp.tile([128, HALF], f32, tag=f"mb{n}", name=f"mb{n}")
                nc.sync.dma_start(out=t, in_=maskbc[n])
                mb.append(t)

            # ---------- MFVI iterations ----------
            # shift order per S-tile: the first matmul of each accumulation
            # group must cover the full 512-column range of its PSUM tile.
            # mats: 0,1 = left (T1,T2: shift -1,-2); 2,3 = right (T1^T,T2^T:
            # shift +1,+2)
            # PE warm-up: ~4us of dummy matmuls so HAM unthrottles the
            # clock before the first real colsum/conv arrives.
            pwarm = psp.tile([128, S], f32, tag="z", name="pwarm", bufs=1)
            for k in range(20):
                nc.tensor.matmul(pwarm[:, 0:128], ones_m[:], ones_m[:],
                                 start=True, stop=True)

            shifts = [(0, -1), (1, -2), (2, +1), (3, +2)]
            NSTEP = ITERS * BPC

            def emit_exp(step):
                it, n = divmod(step, BPC)
                for c in range(NCH):
                    nc.scalar.activation(
                        out=qsm[n][c][:, WINDOW:S + WINDOW],
                        in_=q[n][c][:],
                        func=mybir.ActivationFunctionType.Exp)

            def emit_zchain(step):
                it, n = divmod(step, BPC)
                pz = psp.tile([128, S], f32, tag="z", name=f"z_{it}_{n}", bufs=1)
                for c in range(NCH):
                    for h in range(2):
                        nc.tensor.matmul(
                            pz[:, h * HALF:(h + 1) * HALF],
                            ones_m[:],
                            qsm[n][c][:, WINDOW + h * HALF:
                                      WINDOW + (h + 1) * HALF],
                            start=(c == 0), stop=(c == NCH - 1))
                rb = wp.tile([128, S], f32, tag="rb", name=f"rb_{it}_{n}")
                nc.vector.reciprocal(rb[:], pz[:])
                for c in range(NCH):
                    nc.vector.tensor_mul(
                        qsm[n][c][:, WINDOW:S + WINDOW],
                        qsm[n][c][:, WINDOW:S + WINDOW], rb[:])

            def emit_conv(step):
                it, n = divmod(step, BPC)
                for st in range(2):
                    s0 = st * HALF
                    pm = [psp.tile([128, HALF], f32, tag=f"m{c}",
                                   name=f"m_{it}_{n}_{st}_{c}", bufs=3)
                          for c in range(NCH)]
                    nmm = len(shifts) * NCH
                    cnt = [0, 0]
                    for (m, d) in shifts:
                        for kc in range(NCH):
                            lo = WINDOW + s0 + d
                            for c in range(NCH):
                                i = (m * NCH + kc) * NCH + c
                                nc.tensor.matmul(
                                    pm[c][:],
                                    wmm[:, i * 128:(i + 1) * 128],
                                    qsm[n][kc][:, lo:lo + HALF],
                                    start=(cnt[c] == 0),
                                    stop=(cnt[c] == nmm - 1))
                                cnt[c] += 1
                    for c in range(NCH):
                        # PSUM evacuation must be DVE (GpSimd cannot touch
                        # PSUM); masks are SBUF-only and go to GpSimd.
                        if it == ITERS - 1:
                            # final iteration: q is dead afterwards, so land
                            # results in scratch tiles (no WAR on q) and ship
                            # each half as soon as it is finished.
                            qf = wp.tile([128, HALF], f32, tag=f"qf{st}_{c}",
                                         name=f"qf_{n}_{st}_{c}", bufs=4)
                            nc.vector.tensor_add(
                                out=qf[:], in0=pm[c][:],
                                in1=up[n][c][:, s0:s0 + HALF])
                            if st == 1:
                                # last batch: keep the whole tail chain on
                                # DVE; earlier batches offload to GpSimd
                                meng = (nc.vector if n == BPC - 1
                                        else nc.gpsimd)
                                meng.tensor_mul(
                                    out=qf[:], in0=qf[:], in1=mb[n][:])
                            nc.sync.dma_start(
                                out=qout[n, c][:, s0:s0 + HALF], in_=qf[:])
                        else:
                            nc.vector.tensor_add(
                                out=q[n][c][:, s0:s0 + HALF],
                                in0=pm[c][:], in1=up[n][c][:, s0:s0 + HALF])
                            if st == 1:
                                nc.gpsimd.tensor_mul(
                                    out=q[n][c][:, HALF:],
                                    in0=q[n][c][:, HALF:], in1=mb[n][:])

            # 1-step software pipeline: Z/softmax of step s+1 is emitted
            # before the conv of step s, so the in-order PE queue can fill
            # the normalize wait of step s with colsum work of step s+1.
            emit_exp(0)
            for step in range(NSTEP + 1):
                if step + 1 < NSTEP:
                    emit_exp(step + 1)
                if step < NSTEP:
                    emit_zchain(step)
                if step >= 1:
                    emit_conv(step - 1)

    _split_sync_waits(nc)
    return nc


_CACHED_NC = None


def _get_nc():
    global _CACHED_NC
    if _CACHED_NC is None:
        _CACHED_NC = _build()
    return _CACHED_NC


def _host_prep(token_feats, unary_score, mask, transitions, start_transitions,
               end_transitions, lengths):
    mask = np.asarray(mask, np.float32)
    unary_score = np.asarray(unary_score, np.float32)
    transitions = np.asarray(transitions, np.float32)
    start_transitions = np.asarray(start_transitions, np.float32)
    end_transitions = np.asarray(end_transitions, np.float32)
    lengths = np.asarray(lengths).astype(np.int64)

    unary = unary_score * mask[..., None]                      # [B,S,T]
    ucorr = unary.copy()
    ucorr[:, 0:WINDOW, :] += start_transitions[None, :, :]
    bidx = np.arange(B)
    for j in range(1, WINDOW + 1):
        ucorr[bidx, lengths - j, :] += end_transitions[j - 1][None, :]

    def to_t(x):  # [B,S,T] -> [B, NCH, 128, S]
        return np.ascontiguousarray(
            x.transpose(0, 2, 1).reshape(B, NCH, 128, S))

    u_rawT = to_t(unary)
    u_corrT = to_t(ucorr)

    # lhsT weight blocks: i = (m*NCH + kc)*NCH + mc
    mats = [transitions[0], transitions[1],
            transitions[0].T, transitions[1].T]
    wblk = np.empty((4 * NCH * NCH, 128, 128), np.float32)
    for m in range(4):
        for kc in range(NCH):
            for mc in range(NCH):
                wblk[(m * NCH + kc) * NCH + mc] = \
                    mats[m][kc * 128:(kc + 1) * 128, mc * 128:(mc + 1) * 128]
    # device layout: [128 partitions(k-within-chunk), 16 blocks x 128]
    wblk = np.ascontiguousarray(
        wblk.transpose(1, 0, 2).reshape(128, 4 * NCH * NCH * 128))
    return u_rawT, u_corrT, wblk, mask


def kernel(token_feats, unary_score, mask, transitions, start_transitions,
           end_transitions, lengths):
    u_rawT, u_corrT, wblk, maskf = _host_prep(
        token_feats, unary_score, mask, transitions, start_transitions,
        end_transitions, lengths)

    in_maps = []
    for core in range(N_CORES):
        sl = slice(core * BPC, (core + 1) * BPC)
        in_maps.append({
            "u_raw": np.ascontiguousarray(u_rawT[sl]),
            "u_corr": np.ascontiguousarray(u_corrT[sl]),
            "wmat": wblk,
            "maskbc": np.ascontiguousarray(
                np.broadcast_to(maskf[sl, None, HALF:],
                                (BPC, 128, HALF)).astype(np.float32)),
        })

    nc = _get_nc()
    res = bass_utils.run_bass_kernel_spmd(nc, in_maps,
                                          core_ids=list(range(N_CORES)))
    qT = np.concatenate([res.results[c]["qout"] for c in range(N_CORES)],
                        axis=0)                                # [B,NCH,128,S]
    q = qT.reshape(B, T, S).transpose(0, 2, 1)                 # [B,S,T]
    return np.ascontiguousarray(q.astype(np.float32))



# revision 4
# speedup vs baseline: 1.0382x; 1.0382x over previous
"""Trainium2 Bass kernel for the MFVI second-order CRF message-passing module.

Math (per batch element, per iteration):
    q_sm = softmax(q, axis=-1)                               # over T=256
    msg[s] = q_sm[s-1]@T1 + q_sm[s-2]@T2 + q_sm[s+1]@T1' + q_sm[s+2]@T2'
    q    = (unary + msg + start/end-corrections) * mask

Strategy (v2) - 8 cores, data-parallel over batch (4 elems/core):
  * Length-aware column skipping: batch elems are sorted by length and
    dealt round-robin into BPC=4 "slots" so every core's slot-n elem has a
    similar length; the program only computes columns [0, C_n) where
    C_n = max length in slot n (~3.4k cols/core vs 4k full).  Tail senders
    beyond each elem's true length L are uniform-softmax constants; their
    message contributions are folded into the corrected unary on the host
    and the on-device softmax columns [L, C_n) are zeroed via a masked
    reciprocal, so junk there never propagates.
  * Everything is scaled by 64 on host (u*64) so the softmax operand can be
    quantized to fp8e4 with all values in the normal range; exp() uses
    scale=1/64 to undo it, and the colsum "ones" matrix holds 1/64 so the
    DVE reciprocal directly yields rb = 64/z.
  * Conv matmuls run in fp8e4 DoubleRow mode: both 128-row chunks of the
    T=256 contraction are processed in a single pass at 0.5 cycles/column
    (4x the fp32r rate).  Weights [128, 2, 128] per (tap, out-chunk).
  * The corrected-unary add is done ON THE PE via an fp16 identity matmul
    that joins each PSUM accumulation group (start=True), so the Act engine
    can read exp() straight out of PSUM and DVE only does recip + half the
    normalize (other half on GpSimd).  Final iteration skips the identity
    and lets DVE fuse the add into the PSUM evacuation.
  * Each elem is split into two column halves ("velems") so conv PSUM
    tiles stay <= 2 banks; 24 pipeline steps with a 2-round software
    lookahead keep the PE queue saturated.
  * All HBM I/O is fp16 (64*unary, 64*ucorr in; 64*q out, /64 on host).
"""
import os
import sys

sys.path.insert(0, "/opt/trn_rl_repo")

import numpy as np
import ml_dtypes

import concourse.mybir as mybir
from concourse.bass import Bass
from concourse.tile import TileContext
from concourse import bass_utils

B, S, T = 32, 1024, 256
WINDOW = 2
ITERS = 3
N_CORES = 8
BPC = B // N_CORES          # batch elems per core (= slots)
NCH = T // 128              # partition chunks of T
SCALE = 64.0

NODR = bool(os.environ.get("MFVI_NODR"))    # fallback: bf16 conv, no DoubleRow
NOGP = bool(os.environ.get("MFVI_NOGP"))    # fallback: normalize fully on DVE
WARMUP = 28

# taps: (mat index, shift) ; mats = [T1, T2, T1^T, T2^T]
SHIFTS = [(0, -1), (1, -2), (2, +1), (3, +2)]


def _split_sync_waits(nc):
    """walrus accepts at most ONE sync wait per instruction; Tile emits
    several. Move extras onto same-engine NoOps."""
    ctr = 0
    for f in nc.m.functions:
        for block in f.blocks:
            out = []
            changed = False
            for inst in block.instructions:
                si = inst.sync_info
                waits = list(si.on_wait) if si is not None and si.on_wait else []
                if len(waits) > 1:
                    changed = True
                    for w in waits[:-1]:
                        ctr += 1
                        nop = mybir.InstNoOp(
                            name=f"I-waitsplit-{ctr}",
                            engine=inst.engine, ins=[], outs=[])
                        nop.sync_info = mybir.SyncInfo(on_wait=[w], on_update=[])
                        out.append(nop)
                    si.on_wait = [waits[-1]]
                    inst.sync_info = si
                out.append(inst)
            if changed:
                block.instructions = out
    return nc


def _plan(lengths):
    L = np.asarray(lengths, np.int64)
    order = np.argsort(-L, kind="stable")
    slots = [order[N_CORES * n:N_CORES * (n + 1)] for n in range(BPC)]
    C, F = [], []
    for n in range(BPC):
        ls = L[slots[n]]
        C.append(min(S, int(-(-int(ls.max()) // 16) * 16)))
        F.append(int(ls.min()))
    Ch = [c // 2 for c in C]
    M = [C[n] - F[n] for n in range(BPC)]
    return dict(order=order, slots=slots, C=C, F=F, Ch=Ch, M=M, L=L)


def _build(C, F, Ch, M):
    f32 = mybir.dt.float32
    f16 = mybir.dt.float16
    bf16 = mybir.dt.bfloat16
    f8 = mybir.dt.float8e4
    qdt = bf16 if NODR else f8
    DR = None if NODR else mybir.MatmulPerfMode.DoubleRow

    nc = Bass(trn_type="TRN2", target_bir_lowering=False, debug=False,
              num_devices=N_CORES)

    u_d = [nc.dram_tensor(f"u{n}", [NCH, 128, C[n]], f16,
                          kind="ExternalInput").ap() for n in range(BPC)]
    uc_d = [nc.dram_tensor(f"uc{n}", [NCH, 128, C[n]], f16,
                           kind="ExternalInput").ap() for n in range(BPC)]
    q_d = [nc.dram_tensor(f"q{n}", [NCH, 128, C[n]], f16,
                          kind="ExternalOutput").ap() for n in range(BPC)]
    # DR: 8 blocks of [128, 2, 128] fp8 ; NODR: 16 blocks of [128, 128] bf16
    w_d = nc.dram_tensor("wdr", [128, 2048], qdt, kind="ExternalInput").ap()
    id_d = nc.dram_tensor("ident", [128, 128], f16, kind="ExternalInput").ap()
    on_d = nc.dram_tensor("onesb", [128, 128], bf16, kind="ExternalInput").ap()
    mk_d = [nc.dram_tensor(f"mk{n}", [128, M[n]], f32,
                           kind="ExternalInput").ap() if M[n] > 0 else None
            for n in range(BPC)]

    with TileContext(nc) as tc:
        with tc.tile_pool(name="persist", bufs=1) as pp, \
             tc.tile_pool(name="ebfp", bufs=3) as ebfp, \
             tc.tile_pool(name="rbp", bufs=3) as rbp, \
             tc.tile_pool(name="psum", bufs=3, space="PSUM") as psp:

            u_t = [pp.tile([128, NCH * C[n]], f16, tag=f"u{n}", name=f"u{n}")
                   for n in range(BPC)]
            uc_t = [pp.tile([128, NCH * C[n]], f16, tag=f"uc{n}", name=f"uc{n}")
                    for n in range(BPC)]
            qf_t = [pp.tile([128, NCH * C[n]], f16, tag=f"qf{n}", name=f"qf{n}")
                    for n in range(BPC)]
            qs_t = [pp.tile([128, NCH * (C[n] + 2 * WINDOW)], qdt,
                            tag=f"qs{n}", name=f"qs{n}") for n in range(BPC)]
            w_t = pp.tile([128, 2048], qdt, tag="w", name="w")
            id_t = pp.tile([128, 128], f16, tag="id", name="id")
            on_t = pp.tile([128, 128], bf16, tag="on", name="on")
            mk_t = [pp.tile([128, M[n]], f32, tag=f"mk{n}", name=f"mk{n}")
                    if M[n] > 0 else None for n in range(BPC)]

            # --- input DMA: elems 0,1 + weights now; 2,3 staggered later ---
            def dma_in(n):
                for c in range(NCH):
                    nc.sync.dma_start(out=u_t[n][:, c * C[n]:(c + 1) * C[n]],
                                      in_=u_d[n][c])
                for c in range(NCH):
                    nc.sync.dma_start(out=uc_t[n][:, c * C[n]:(c + 1) * C[n]],
                                      in_=uc_d[n][c])

            dma_in(0)
            nc.sync.dma_start(out=on_t, in_=on_d)
            nc.sync.dma_start(out=w_t, in_=w_d)
            nc.sync.dma_start(out=id_t, in_=id_d)
            for n in range(BPC):
                if mk_t[n] is not None:
                    nc.sync.dma_start(out=mk_t[n], in_=mk_d[n])
            dma_in(1)

            # qsm guard columns (2 cols each side of each chunk) -> 0
            for n in range(BPC):
                row = C[n] + 2 * WINDOW
                for c in range(NCH):
                    nc.gpsimd.memset(qs_t[n][:, c * row:c * row + WINDOW], 0.0)
                    nc.gpsimd.memset(
                        qs_t[n][:, c * row + WINDOW + C[n]:(c + 1) * row], 0.0)

            # PE p-state warmup (~3us of dummy matmuls)
            pwarm = psp.tile([128, 512], f32, tag="z", name="pwarm", bufs=2)
            for k in range(WARMUP):
                nc.tensor.matmul(pwarm[:, 0:128], on_t[:], on_t[:],
                                 start=True, stop=True)

            # ---------------- pipeline ----------------
            NSTEP = BPC * ITERS * 2
            psums = {}

            def sih(x):
                n, r = divmod(x, ITERS * 2)
                it, h = divmod(r, 2)
                return n, it, h

            def emit_exp(x):
                n, it, h = sih(x)
                ch, cn = Ch[n], C[n]
                ebf = ebfp.tile([128, 1024], bf16, tag="ebf", name=f"ebf{x}")
                dst = ebf[:, 0:NCH * ch].rearrange("p (a b) -> p a b", a=NCH)
                if it == 0:
                    src = u_t[n][:, 0:NCH * cn].rearrange(
                        "p (a b) -> p a b", a=NCH)[:, :, h * ch:(h + 1) * ch]
                else:
                    pv = psums[x - 2]
                    src = pv[:, 0:1024].rearrange(
                        "p (a b) -> p a b", a=NCH)[:, :, 0:ch]
                nc.scalar.activation(out=dst, in_=src,
                                     func=mybir.ActivationFunctionType.Exp,
                                     scale=1.0 / SCALE)
                return ebf

            ebfs = {}

            def emit_zchain(x):
                n, it, h = sih(x)
                ch, cn = Ch[n], C[n]
                ebf = ebfs[x]
                zt = psp.tile([128, 512], f32, tag="z", name=f"z{x}", bufs=2)
                for c in range(NCH):
                    nc.tensor.matmul(zt[:, 0:ch], on_t[:],
                                     ebf[:, c * ch:(c + 1) * ch],
                                     start=(c == 0), stop=(c == NCH - 1))
                rb = rbp.tile([128, 512], f32, tag="rb", name=f"rb{x}")
                nc.vector.reciprocal(rb[:, 0:ch], zt[:, 0:ch])
                if h == 1 and M[n] > 0:
                    lo = F[n] - ch
                    nc.vector.tensor_mul(out=rb[:, lo:lo + M[n]],
                                         in0=rb[:, lo:lo + M[n]],
                                         in1=mk_t[n][:])
                row = cn + 2 * WINDOW
                for c in range(NCH):
                    dst = qs_t[n][:, c * row + WINDOW + h * ch:
                                  c * row + WINDOW + (h + 1) * ch]
                    eng = nc.vector if (c == 0 or NOGP) else nc.gpsimd
                    eng.tensor_mul(out=dst, in0=ebf[:, c * ch:(c + 1) * ch],
                                   in1=rb[:, 0:ch])

            def emit_conv(x):
                n, it, h = sih(x)
                ch, cn = Ch[n], C[n]
                row = cn + 2 * WINDOW
                pv = psp.tile([128, 1024], f32, tag="cv", name=f"cv{x}")
                psums[x] = pv
                qsr = qs_t[n][:, 0:NCH * row].rearrange("p (a b) -> p a b",
                                                        a=NCH)
                ucr = uc_t[n][:, 0:NCH * cn].rearrange("p (a b) -> p a b",
                                                       a=NCH)
                half = ch // 2
                blocks = [(0, half), (half, ch - half)] if not NODR \
                    else [(0, ch)]
                base = WINDOW + h * ch
                for mc in range(NCH):
                    for (co, nn) in blocks:
                        out = pv[:, mc * 512 + co:mc * 512 + co + nn]
                        first = True
                        if it < ITERS - 1:
                            o = mc * cn + h * ch + co
                            nc.tensor.matmul(
                                out, id_t[:], uc_t[n][:, o:o + nn],
                                start=True, stop=False)
                            first = False
                        if NODR:
                            nmm = len(SHIFTS) * NCH
                            cnt = 0
                            for (t, d) in SHIFTS:
                                for kt in range(NCH):
                                    lhs = w_t[:, ((t * 2 + mc) * 2 + kt) * 128:
                                              ((t * 2 + mc) * 2 + kt + 1) * 128]
                                    rhs = qs_t[n][:, kt * row + base + co + d:
                                                  kt * row + base + co + d + nn]
                                    nc.tensor.matmul(
                                        out, lhs, rhs,
                                        start=(first and cnt == 0),
                                        stop=(cnt == nmm - 1))
                                    cnt += 1
                        else:
                            for ti, (t, d) in enumerate(SHIFTS):
                                lhs = w_t[:, (t * 2 + mc) * 256:
                                          (t * 2 + mc + 1) * 256].rearrange(
                                    "p (a b) -> p a b", a=2)
                                rhs = qsr[:, :, base + co + d:
                                          base + co + d + nn]
                                nc.tensor.matmul(
                                    out, lhs, rhs,
                                    start=(first and ti == 0),
                                    stop=(ti == len(SHIFTS) - 1),
                                    perf_mode=mybir.MatmulPerfMode.DoubleRow)
                if it == ITERS - 1:
                    pvr = pv[:, 0:1024].rearrange("p (a b) -> p a b",
                                                  a=NCH)[:, :, 0:ch]
                    qfr = qf_t[n][:, 0:NCH * cn].rearrange(
                        "p (a b) -> p a b", a=NCH)[:, :, h * ch:(h + 1) * ch]
                    ucv = ucr[:, :, h * ch:(h + 1) * ch]
                    nc.vector.tensor_add(out=qfr, in0=pvr, in1=ucv)
                    if h == 1:
                        for c in range(NCH):
                            nc.sync.dma_start(
                                out=q_d[n][c],
                                in_=qf_t[n][:, c * cn:(c + 1) * cn])

            # round r emits: zchain(r), conv(r-1), exp(r+1).
            # PE queue per round = [colsum(r), conv(r-1)]: the short colsum
            # slots in front of the long conv so the softmax chain of step r
            # (recip+norm on DVE/GpSimd) completes while conv(r-1) streams.
            ebfs[0] = emit_exp(0)
            for r in range(NSTEP + 1):
                if r == 1:
                    dma_in(2)
                if r == 4:
                    dma_in(3)
                if r < NSTEP:
                    emit_zchain(r)
                if 0 <= r - 1 < NSTEP:
                    emit_conv(r - 1)
                if r + 1 < NSTEP:
                    ebfs[r + 1] = emit_exp(r + 1)

    _split_sync_waits(nc)
    return nc


_CACHE = {}
_LAST_NC = None


def _get_nc(plan=None):
    global _LAST_NC
    if plan is None:
        return _LAST_NC
    key = (tuple(plan["C"]), tuple(plan["F"]))
    if key not in _CACHE:
        _CACHE[key] = _build(plan["C"], plan["F"], plan["Ch"], plan["M"])
    _LAST_NC = _CACHE[key]
    return _LAST_NC


def _host_prep(unary_score, mask, transitions, start_transitions,
               end_transitions, lengths, plan):
    f16 = np.float16
    L = plan["L"]
    unary = (np.asarray(unary_score, np.float32)
             * np.asarray(mask, np.float32)[..., None])      # [B,S,T]
    trans = np.asarray(transitions, np.float32)
    ucorr = unary.copy()
    ucorr[:, 0:WINDOW, :] += np.asarray(start_transitions, np.float32)
    rowmean = [trans[j].sum(axis=1) / T for j in range(WINDOW)]  # u @ Tj^T
    endt = np.asarray(end_transitions, np.float32)
    for b in range(B):
        lb = int(L[b])
        for j in range(1, WINDOW + 1):
            ucorr[b, lb - j, :] += endt[j - 1]
        # uniform-softmax senders in the masked tail [lb, S)
        if lb <= S - 1:
            ucorr[b, lb - 1, :] += rowmean[0]      # sender lb,   j=1
            ucorr[b, lb - 2, :] += rowmean[1]      # sender lb,   j=2
        if lb <= S - 2:
            ucorr[b, lb - 1, :] += rowmean[1]      # sender lb+1, j=2
    mats = [trans[0], trans[1], trans[0].T, trans[1].T]

    qdt = ml_dtypes.bfloat16 if NODR else ml_dtypes.float8_e4m3
    wpk = np.zeros((128, 2048), np.float32)
    for t in range(4):
        for mc in range(NCH):
            for kt in range(NCH):
                blk = mats[t][kt * 128:(kt + 1) * 128,
                              mc * 128:(mc + 1) * 128]       # [kp, m]
                if NODR:
                    o = ((t * 2 + mc) * 2 + kt) * 128
                    wpk[:, o:o + 128] = blk
                else:
                    o = (t * 2 + mc) * 256 + kt * 128
                    wpk[:, o:o + 128] = blk
    wpk = wpk.astype(qdt)
    ident = np.eye(128, dtype=f16)
    onesb = np.full((128, 128), 1.0 / SCALE, ml_dtypes.bfloat16)

    u64 = (unary * SCALE).astype(f16)
    uc64 = (ucorr * SCALE).astype(f16)

    in_maps = []
    for core in range(N_CORES):
        m = {"wdr": wpk, "ident": ident, "onesb": onesb}
        for n in range(BPC):
            b = int(plan["slots"][n][core])
            cn = plan["C"][n]
            m[f"u{n}"] = np.ascontiguousarray(
                u64[b, :cn].T.reshape(NCH, 128, cn))
            m[f"uc{n}"] = np.ascontiguousarray(
                uc64[b, :cn].T.reshape(NCH, 128, cn))
            if plan["M"][n] > 0:
                lb = int(L[b])
                col = np.arange(plan["F"][n], cn) < lb
                m[f"mk{n}"] = np.ascontiguousarray(
                    np.broadcast_to(col[None, :].astype(np.float32),
                                    (128, plan["M"][n])))
        in_maps.append(m)
    return in_maps


def kernel(token_feats, unary_score, mask, transitions, start_transitions,
           end_transitions, lengths):
    plan = _plan(lengths)
    nc = _get_nc(plan)
    in_maps = _host_prep(unary_score, mask, transitions, start_transitions,
                         end_transitions, lengths, plan)
    res = bass_utils.run_bass_kernel_spmd(nc, in_maps,
                                          core_ids=list(range(N_CORES)))
    out = np.zeros((B, S, T), np.float32)
    L = plan["L"]
    for core in range(N_CORES):
        for n in range(BPC):
            b = int(plan["slots"][n][core])
            cn = plan["C"][n]
            qv = np.asarray(res.results[core][f"q{n}"],
                            np.float32)                       # [2,128,cn]
            lb = int(L[b])
            out[b, :lb, :] = qv.reshape(T, cn).T[:lb] / SCALE
    return out


# revision 7
# speedup vs baseline: 1.8761x; 1.8071x over previous
"""Trainium2 Bass kernel for the MFVI second-order CRF message-passing module.

Math (per batch element, per iteration):
    q_sm = softmax(q, axis=-1)                               # over T=256
    msg[s] = q_sm[s-1]@T1 + q_sm[s-2]@T2 + q_sm[s+1]@T1' + q_sm[s+2]@T2'
    q    = (unary + msg + start/end-corrections) * mask

Strategy (v2) - 8 cores, data-parallel over batch (4 elems/core):
  * Length-aware column skipping: batch elems are sorted by length and
    dealt round-robin into BPC=4 "slots" so every core's slot-n elem has a
    similar length; the program only computes columns [0, C_n) where
    C_n = max length in slot n (~3.4k cols/core vs 4k full).  Tail senders
    beyond each elem's true length L are uniform-softmax constants; their
    message contributions are folded into the corrected unary on the host
    and the on-device softmax columns [L, C_n) are zeroed via a masked
    reciprocal, so junk there never propagates.
  * Everything is scaled by 64 on host (u*64) so the softmax operand can be
    quantized to fp8e4 with all values in the normal range; exp() uses
    scale=1/64 to undo it, and the colsum "ones" matrix holds 1/64 so the
    DVE reciprocal directly yields rb = 64/z.
  * Conv matmuls run in fp8e4 DoubleRow mode: both 128-row chunks of the
    T=256 contraction are processed in a single pass at 0.5 cycles/column
    (4x the fp32r rate).  Weights [128, 2, 128] per (tap, out-chunk).
  * The corrected-unary add is done ON THE PE via an fp16 identity matmul
    that joins each PSUM accumulation group (start=True), so the Act engine
    can read exp() straight out of PSUM and DVE only does recip + half the
    normalize (other half on GpSimd).  Final iteration skips the identity
    and lets DVE fuse the add into the PSUM evacuation.
  * Each elem is split into two column halves ("velems") so conv PSUM
    tiles stay <= 2 banks; 24 pipeline steps with a 2-round software
    lookahead keep the PE queue saturated.
  * All HBM I/O is fp16 (64*unary, 64*ucorr in; 64*q out, /64 on host).
"""
import os
import sys

sys.path.insert(0, "/opt/trn_rl_repo")

import numpy as np
import ml_dtypes

import concourse.mybir as mybir
from concourse.bass import Bass
from concourse.tile import TileContext
from concourse import bass_utils

B, S, T = 32, 1024, 256
WINDOW = 2
ITERS = 3
N_CORES = 8
BPC = B // N_CORES          # batch elems per core (= slots)
NCH = T // 128              # partition chunks of T
SCALE = 64.0

NODR = bool(os.environ.get("MFVI_NODR"))    # fallback: bf16 conv, no DoubleRow
NOGP = bool(os.environ.get("MFVI_NOGP"))    # fallback: normalize fully on DVE
WARMUP = 28

# taps: (mat index, shift) ; mats = [T1, T2, T1^T, T2^T]
SHIFTS = [(0, -1), (1, -2), (2, +1), (3, +2)]


def _split_sync_waits(nc):
    """walrus accepts at most ONE sync wait per instruction; Tile emits
    several. Move extras onto same-engine NoOps."""
    ctr = 0
    for f in nc.m.functions:
        for block in f.blocks:
            out = []
            changed = False
            for inst in block.instructions:
                si = inst.sync_info
                waits = list(si.on_wait) if si is not None and si.on_wait else []
                if len(waits) > 1:
                    changed = True
                    for w in waits[:-1]:
                        ctr += 1
                        nop = mybir.InstNoOp(
                            name=f"I-waitsplit-{ctr}",
                            engine=inst.engine, ins=[], outs=[])
                        nop.sync_info = mybir.SyncInfo(on_wait=[w], on_update=[])
                        out.append(nop)
                    si.on_wait = [waits[-1]]
                    inst.sync_info = si
                out.append(inst)
            if changed:
                block.instructions = out
    return nc


def _plan(lengths):
    L = np.asarray(lengths, np.int64)
    order = np.argsort(-L, kind="stable")
    slots = [order[N_CORES * n:N_CORES * (n + 1)] for n in range(BPC)]
    C, F = [], []
    for n in range(BPC):
        ls = L[slots[n]]
        C.append(min(S, int(-(-int(ls.max()) // 16) * 16)))
        F.append(int(ls.min()))
    Ch = [c // 2 for c in C]
    M = [C[n] - F[n] for n in range(BPC)]
    return dict(order=order, slots=slots, C=C, F=F, Ch=Ch, M=M, L=L)


def _build(C, F, Ch, M):
    f32 = mybir.dt.float32
    f16 = mybir.dt.float16
    bf16 = mybir.dt.bfloat16
    f8 = mybir.dt.float8e4
    qdt = bf16 if NODR else f8
    DR = None if NODR else mybir.MatmulPerfMode.DoubleRow

    nc = Bass(trn_type="TRN2", target_bir_lowering=False, debug=False,
              num_devices=N_CORES)

    u_d = [nc.dram_tensor(f"u{n}", [NCH, 128, C[n]], f16,
                          kind="ExternalInput").ap() for n in range(BPC)]
    uc_d = [nc.dram_tensor(f"uc{n}", [NCH, 128, C[n]], f16,
                           kind="ExternalInput").ap() for n in range(BPC)]
    q_d = [nc.dram_tensor(f"q{n}", [NCH, 128, C[n]], f16,
                          kind="ExternalOutput").ap() for n in range(BPC)]
    # DR: 8 blocks of [128, 2, 128] fp8 ; NODR: 16 blocks of [128, 128] bf16
    w_d = nc.dram_tensor("wdr", [128, 2048], qdt, kind="ExternalInput").ap()
    id_d = nc.dram_tensor("ident", [128, 128], f16, kind="ExternalInput").ap()
    on_d = nc.dram_tensor("onesb", [128, 128], bf16, kind="ExternalInput").ap()
    mk_d = [nc.dram_tensor(f"mk{n}", [128, M[n]], f32,
                           kind="ExternalInput").ap() if M[n] > 0 else None
            for n in range(BPC)]

    with TileContext(nc) as tc:
        with tc.tile_pool(name="persist", bufs=1) as pp, \
             tc.tile_pool(name="ebfp", bufs=3) as ebfp, \
             tc.tile_pool(name="rbp", bufs=3) as rbp, \
             tc.tile_pool(name="psum", bufs=3, space="PSUM") as psp:

            u_t = [pp.tile([128, NCH * C[n]], f16, tag=f"u{n}", name=f"u{n}")
                   for n in range(BPC)]
            uc_t = [pp.tile([128, NCH * C[n]], f16, tag=f"uc{n}", name=f"uc{n}")
                    for n in range(BPC)]
            qf_t = [pp.tile([128, NCH * C[n]], f16, tag=f"qf{n}", name=f"qf{n}")
                    for n in range(BPC)]
            qs_t = [pp.tile([128, NCH * (C[n] + 2 * WINDOW)], qdt,
                            tag=f"qs{n}", name=f"qs{n}") for n in range(BPC)]
            w_t = pp.tile([128, 2048], qdt, tag="w", name="w")
            id_t = pp.tile([128, 128], f16, tag="id", name="id")
            on_t = pp.tile([128, 128], bf16, tag="on", name="on")
            mk_t = [pp.tile([128, M[n]], f32, tag=f"mk{n}", name=f"mk{n}")
                    if M[n] > 0 else None for n in range(BPC)]

            # --- input DMA: elems 0,1 + weights now; 2,3 staggered later ---
            def dma_in(n):
                for c in range(NCH):
                    nc.sync.dma_start(out=u_t[n][:, c * C[n]:(c + 1) * C[n]],
                                      in_=u_d[n][c])
                for c in range(NCH):
                    nc.sync.dma_start(out=uc_t[n][:, c * C[n]:(c + 1) * C[n]],
                                      in_=uc_d[n][c])

            dma_in(0)
            nc.sync.dma_start(out=on_t, in_=on_d)
            nc.sync.dma_start(out=w_t, in_=w_d)
            nc.sync.dma_start(out=id_t, in_=id_d)
            for n in range(BPC):
                if mk_t[n] is not None:
                    nc.sync.dma_start(out=mk_t[n], in_=mk_d[n])
            dma_in(1)

            # qsm guard columns (2 cols each side of each chunk) -> 0
            for n in range(BPC):
                row = C[n] + 2 * WINDOW
                for c in range(NCH):
                    nc.gpsimd.memset(qs_t[n][:, c * row:c * row + WINDOW], 0.0)
                    nc.gpsimd.memset(
                        qs_t[n][:, c * row + WINDOW + C[n]:(c + 1) * row], 0.0)

            # PE p-state warmup (~3us of dummy matmuls)
            pwarm = psp.tile([128, 512], f32, tag="z", name="pwarm", bufs=2)
            for k in range(WARMUP):
                nc.tensor.matmul(pwarm[:, 0:128], on_t[:], on_t[:],
                                 start=True, stop=True)

            # ---------------- pipeline ----------------
            # Steps interleave TWO elems per iteration (lanes e0A,e0B,e1A,e1B)
            # so the per-velem softmax latency chain (conv -> exp -> colsum ->
            # recip -> norm -> conv, ~3.5us) overlaps with ~4 PE rounds of
            # other lanes' work.  exp(x) consumes the conv PSUM of x-4.
            NSTEP = BPC * ITERS * 2
            STEPS = []
            for pair in range(BPC // 2):
                for it in range(ITERS):
                    for e in (2 * pair, 2 * pair + 1):
                        for h in range(2):
                            STEPS.append((e, it, h))
            psums = {}

            def sih(x):
                return STEPS[x]

            def emit_exp(x):
                n, it, h = sih(x)
                ch, cn = Ch[n], C[n]
                ebf = ebfp.tile([128, 1024], bf16, tag="ebf", name=f"ebf{x}")
                dst = ebf[:, 0:NCH * ch].rearrange("p (a b) -> p a b", a=NCH)
                if it == 0:
                    src = u_t[n][:, 0:NCH * cn].rearrange(
                        "p (a b) -> p a b", a=NCH)[:, :, h * ch:(h + 1) * ch]
                else:
                    pv = psums[x - 4]
                    src = pv[:, 0:1024].rearrange(
                        "p (a b) -> p a b", a=NCH)[:, :, 0:ch]
                nc.scalar.activation(out=dst, in_=src,
                                     func=mybir.ActivationFunctionType.Exp,
                                     scale=1.0 / SCALE)
                return ebf

            ebfs = {}

            def emit_zchain(x):
                n, it, h = sih(x)
                ch, cn = Ch[n], C[n]
                ebf = ebfs[x]
                zt = psp.tile([128, 512], f32, tag="z", name=f"z{x}", bufs=2)
                for c in range(NCH):
                    nc.tensor.matmul(zt[:, 0:ch], on_t[:],
                                     ebf[:, c * ch:(c + 1) * ch],
                                     start=(c == 0), stop=(c == NCH - 1))
                rb = rbp.tile([128, 512], f32, tag="rb", name=f"rb{x}")
                nc.vector.reciprocal(rb[:, 0:ch], zt[:, 0:ch])
                if h == 1 and M[n] > 0:
                    lo = F[n] - ch
                    nc.vector.tensor_mul(out=rb[:, lo:lo + M[n]],
                                         in0=rb[:, lo:lo + M[n]],
                                         in1=mk_t[n][:])
                row = cn + 2 * WINDOW
                for c in range(NCH):
                    dst = qs_t[n][:, c * row + WINDOW + h * ch:
                                  c * row + WINDOW + (h + 1) * ch]
                    eng = nc.vector if (c == 0 or NOGP) else nc.gpsimd
                    eng.tensor_mul(out=dst, in0=ebf[:, c * ch:(c + 1) * ch],
                                   in1=rb[:, 0:ch])

            def emit_conv(x):
                n, it, h = sih(x)
                ch, cn = Ch[n], C[n]
                row = cn + 2 * WINDOW
                pv = psp.tile([128, 1024], f32, tag="cv", name=f"cv{x}")
                psums[x] = pv
                qsr = qs_t[n][:, 0:NCH * row].rearrange("p (a b) -> p a b",
                                                        a=NCH)
                ucr = uc_t[n][:, 0:NCH * cn].rearrange("p (a b) -> p a b",
                                                       a=NCH)
                half = ch // 2
                blocks = [(0, half), (half, ch - half)] if not NODR \
                    else [(0, ch)]
                base = WINDOW + h * ch
                for mc in range(NCH):
                    for (co, nn) in blocks:
                        out = pv[:, mc * 512 + co:mc * 512 + co + nn]
                        first = True
                        if it < ITERS - 1:
                            o = mc * cn + h * ch + co
                            nc.tensor.matmul(
                                out, id_t[:], uc_t[n][:, o:o + nn],
                                start=True, stop=False)
                            first = False
                        if NODR:
                            nmm = len(SHIFTS) * NCH
                            cnt = 0
                            for (t, d) in SHIFTS:
                                for kt in range(NCH):
                                    lhs = w_t[:, ((t * 2 + mc) * 2 + kt) * 128:
                                              ((t * 2 + mc) * 2 + kt + 1) * 128]
                                    rhs = qs_t[n][:, kt * row + base + co + d:
                                                  kt * row + base + co + d + nn]
                                    nc.tensor.matmul(
                                        out, lhs, rhs,
                                        start=(first and cnt == 0),
                                        stop=(cnt == nmm - 1))
                                    cnt += 1
                        else:
                            for ti, (t, d) in enumerate(SHIFTS):
                                lhs = w_t[:, (t * 2 + mc) * 256:
                                          (t * 2 + mc + 1) * 256].rearrange(
                                    "p (a b) -> p a b", a=2)
                                rhs = qsr[:, :, base + co + d:
                                          base + co + d + nn]
                                nc.tensor.matmul(
                                    out, lhs, rhs,
                                    start=(first and ti == 0),
                                    stop=(ti == len(SHIFTS) - 1),
                                    perf_mode=mybir.MatmulPerfMode.DoubleRow)
                if it == ITERS - 1:
                    pvr = pv[:, 0:1024].rearrange("p (a b) -> p a b",
                                                  a=NCH)[:, :, 0:ch]
                    qfr = qf_t[n][:, 0:NCH * cn].rearrange(
                        "p (a b) -> p a b", a=NCH)[:, :, h * ch:(h + 1) * ch]
                    ucv = ucr[:, :, h * ch:(h + 1) * ch]
                    nc.vector.tensor_add(out=qfr, in0=pvr, in1=ucv)
                    if h == 1:
                        for c in range(NCH):
                            nc.sync.dma_start(
                                out=q_d[n][c],
                                in_=qf_t[n][:, c * cn:(c + 1) * cn])

            # round r emits: zchain(r), conv(r-2), exp(r+2).
            # PE queue per round = [colsum(r), conv(r-2)]: conv trails its
            # own norm by 2 rounds and the seam-partner norm by 1 round, so
            # the PE never waits; exp(r+2) is emitted right after conv(r-2)
            # (whose PSUM it reads), keeping <=3 conv PSUMs alive.
            ebfs[0] = emit_exp(0)
            ebfs[1] = emit_exp(1)
            for r in range(NSTEP + 2):
                if r == 4:
                    dma_in(2)
                if r == 6:
                    dma_in(3)
                if r < NSTEP:
                    emit_zchain(r)
                if 0 <= r - 2 < NSTEP:
                    emit_conv(r - 2)
                if r + 2 < NSTEP:
                    ebfs[r + 2] = emit_exp(r + 2)

    _split_sync_waits(nc)
    return nc


_CACHE = {}
_LAST_NC = None


def _get_nc(plan=None):
    global _LAST_NC
    if plan is None:
        return _LAST_NC
    key = (tuple(plan["C"]), tuple(plan["F"]))
    if key not in _CACHE:
        _CACHE[key] = _build(plan["C"], plan["F"], plan["Ch"], plan["M"])
    _LAST_NC = _CACHE[key]
    return _LAST_NC


def _host_prep(unary_score, mask, transitions, start_transitions,
               end_transitions, lengths, plan):
    f16 = np.float16
    L = plan["L"]
    unary = (np.asarray(unary_score, np.float32)
             * np.asarray(mask, np.float32)[..., None])      # [B,S,T]
    trans = np.asarray(transitions, np.float32)
    ucorr = unary.copy()
    ucorr[:, 0:WINDOW, :] += np.asarray(start_transitions, np.float32)
    rowmean = [trans[j].sum(axis=1) / T for j in range(WINDOW)]  # u @ Tj^T
    endt = np.asarray(end_transitions, np.float32)
    for b in range(B):
        lb = int(L[b])
        for j in range(1, WINDOW + 1):
            ucorr[b, lb - j, :] += endt[j - 1]
        # uniform-softmax senders in the masked tail [lb, S)
        if lb <= S - 1:
            ucorr[b, lb - 1, :] += rowmean[0]      # sender lb,   j=1
            ucorr[b, lb - 2, :] += rowmean[1]      # sender lb,   j=2
        if lb <= S - 2:
            ucorr[b, lb - 1, :] += rowmean[1]      # sender lb+1, j=2
    mats = [trans[0], trans[1], trans[0].T, trans[1].T]

    qdt = ml_dtypes.bfloat16 if NODR else ml_dtypes.float8_e4m3
    wpk = np.zeros((128, 2048), np.float32)
    for t in range(4):
        for mc in range(NCH):
            for kt in range(NCH):
                blk = mats[t][kt * 128:(kt + 1) * 128,
                              mc * 128:(mc + 1) * 128]       # [kp, m]
                if NODR:
                    o = ((t * 2 + mc) * 2 + kt) * 128
                    wpk[:, o:o + 128] = blk
                else:
                    o = (t * 2 + mc) * 256 + kt * 128
                    wpk[:, o:o + 128] = blk
    wpk = wpk.astype(qdt)
    ident = np.eye(128, dtype=f16)
    onesb = np.full((128, 128), 1.0 / SCALE, ml_dtypes.bfloat16)

    u64 = (unary * SCALE).astype(f16)
    uc64 = (ucorr * SCALE).astype(f16)

    in_maps = []
    for core in range(N_CORES):
        m = {"wdr": wpk, "ident": ident, "onesb": onesb}
        for n in range(BPC):
            b = int(plan["slots"][n][core])
            cn = plan["C"][n]
            m[f"u{n}"] = np.ascontiguousarray(
                u64[b, :cn].T.reshape(NCH, 128, cn))
            m[f"uc{n}"] = np.ascontiguousarray(
                uc64[b, :cn].T.reshape(NCH, 128, cn))
            if plan["M"][n] > 0:
                lb = int(L[b])
                col = np.arange(plan["F"][n], cn) < lb
                m[f"mk{n}"] = np.ascontiguousarray(
                    np.broadcast_to(col[None, :].astype(np.float32),
                                    (128, plan["M"][n])))
        in_maps.append(m)
    return in_maps


def kernel(token_feats, unary_score, mask, transitions, start_transitions,
           end_transitions, lengths):
    plan = _plan(lengths)
    nc = _get_nc(plan)
    in_maps = _host_prep(unary_score, mask, transitions, start_transitions,
                         end_transitions, lengths, plan)
    res = bass_utils.run_bass_kernel_spmd(nc, in_maps,
                                          core_ids=list(range(N_CORES)))
    out = np.zeros((B, S, T), np.float32)
    L = plan["L"]
    for core in range(N_CORES):
        for n in range(BPC):
            b = int(plan["slots"][n][core])
            cn = plan["C"][n]
            qv = np.asarray(res.results[core][f"q{n}"],
                            np.float32)                       # [2,128,cn]
            lb = int(L[b])
            out[b, :lb, :] = qv.reshape(T, cn).T[:lb] / SCALE
    return out
